# revision 1
# baseline (speedup 1.0000x reference)
"""Trainium2 Bass kernel for nn_ConceptFusionModule.

Math: the reference broadcasts a per-batch (B, D) fused vector over the N
sequence positions *before* rmsnorm + out-projection, so the big
(B, N, D) @ (D, D) matmul is rank-deficient: every row of its output is
identical per batch.  The whole module collapses to

    logits  = textN @ Wc.T                # (B*L, C)
    cw      = softmax(logits, -1)
    crT     = textN_b.T @ cw_b            # (D, C) per batch
    v       = crT.T @ Wv.T                # (B*C, D)
    fused_b = blend@v_b + sig(g)*.3*var_c(v_b)       # (B, D)
    y       = rmsnorm(fused) * nw
    obd     = y @ Wo.T                    # (B, D)
    out     = x + obd[:, None, :]

Wq/Wk cancel (softmax over a single key) and are never loaded.

Sharding: the only O(B*N*D) work is the final broadcast add, which is
data-parallel: each of the 8 cores gets 512 rows of each batch of x.
The small chain (everything above the final add) is replicated on every
core — no collectives.  Host-side work is layout only (slicing /
transposition of inputs); every FLOP of the module runs on device.
"""

import os

import numpy as np

import concourse.bacc as bacc
import concourse.bass as bass
import concourse.mybir as mybir
import concourse.tile as tile
from concourse import masks
from concourse.bass_utils import run_bass_kernel_spmd

F32 = mybir.dt.float32
F32R = mybir.dt.float32r

N_CORES = 8
B, N, L, D, C = 2, 4096, 256, 2048, 6
BL = B * L              # 512 text rows
ROWS = B * N // N_CORES  # 1024 x-rows per core
HALF = ROWS // 2         # 512 rows from each batch
KT = D // 128            # 16 contraction tiles
NCH = D // 512           # 4 free-dim chunks
AX = mybir.AxisListType.X
AF = mybir.ActivationFunctionType

USE_FP32R = os.environ.get("BASS_FP32R", "1") == "1"


# float32r streams one row/cycle (vs 4 for fp32) once the moving dim is
# >=256.  The verifier requires fp32r matmul operands to be *produced* as
# fp32r, so the big matmul-input tiles are allocated in that dtype and the
# (bit-identical) DRAM source APs are bitcast for the load.
MMDT = F32R if USE_FP32R else F32


def _mm(ap):
    return ap


def build_nc(is_surreal: bool) -> bacc.Bacc:
    nc = bacc.Bacc("TRN2", target_bir_lowering=False, debug=False,
                   num_devices=N_CORES)

    x_d = nc.dram_tensor("x_shard", [ROWS, D], F32, kind="ExternalInput")
    tN_d = nc.dram_tensor("textN", [BL, D], F32, kind="ExternalInput")
    tT_d = nc.dram_tensor("textT", [D, BL], F32, kind="ExternalInput")
    wct_d = nc.dram_tensor("WcT", [D, C], F32, kind="ExternalInput")
    wvt_d = nc.dram_tensor("WvT", [D, D], F32, kind="ExternalInput")
    wot_d = nc.dram_tensor("WoT", [D, D], F32, kind="ExternalInput")
    bl_d = nc.dram_tensor("blend", [1, C], F32, kind="ExternalInput")
    sg_d = nc.dram_tensor("sg2", [2, 1], F32, kind="ExternalInput")
    nw_d = nc.dram_tensor("nw2", [2, D], F32, kind="ExternalInput")
    out_d = nc.dram_tensor("out_shard", [ROWS, D], F32, kind="ExternalOutput")

    with tile.TileContext(nc) as tc:
        with (
            tc.tile_pool(name="pc", bufs=1) as pc,
            tc.tile_pool(name="pwrk", bufs=2) as pwrk,
        ):
            # ---- constant / activations SBUF ----
            blend_sb = pc.tile([1, C], F32)
            sg_sb = pc.tile([2, 1], F32)
            nw_sb = pc.tile([2, D], F32)
            ident = pc.tile([128, 128], F32)
            sel0 = pc.tile([2, 128], F32)
            sel1 = pc.tile([2, 128], F32)
            cwbd = pc.tile([128, 4, 64], F32)       # block-diag cluster weights (M-padded)
            bd4 = pc.tile([2 * C, 4], F32)          # [blend_b0|blend_b1|1_b0|1_b1]
            ones2 = pc.tile([2 * C, 2], F32)
            crT_sb = pc.tile([128, KT, 64], MMDT)
            cr_sb = pc.tile([2 * C, D], F32)
            yT_sb = pc.tile([128, KT, 64], MMDT)
            lg_sb = pc.tile([C, BL], F32)
            blendn = pc.tile([1, C], F32)
            v_sb = pc.tile([2 * C, D], F32)
            v2_sb = pc.tile([2 * C, D], F32)
            fused_sb = pc.tile([2, D], F32)
            sq_sb = pc.tile([2, D], F32)
            y_sb = pc.tile([2, D], F32)
            obd_sb = pc.tile([2, D], F32)
            bc0 = pc.tile([128, D], F32)
            bc1 = pc.tile([128, D], F32)
            g3 = pc.tile([2, 1], F32)
            ms = pc.tile([2, 1], F32)
            rs = pc.tile([2, 1], F32)
            eps_t = pc.tile([2, 1], F32)
            m12 = pc.tile([2 * C, 1], F32)
            m12c = pc.tile([2 * C, 1], F32)
            blendn2 = pc.tile([1, 2 * C], F32)
            bmx = pc.tile([1, 1], F32)
            bsum = pc.tile([1, 1], F32)
            brcp = pc.tile([1, 1], F32)

            # ---- text pool: freed after the crT phase to make room for Wo/x ----
            ptext_cm = tc.tile_pool(name="ptext", bufs=1)
            ptext = ptext_cm.__enter__()
            textN = ptext.tile([128, 4, D], F32)    # (l-tile, d) natural
            textT = ptext.tile([128, KT, BL], MMDT)  # (d-tile, b*l)
            wct = ptext.tile([128, KT, C], MMDT)

            # ---- phase 0: loads (sync HWDGE ring is FIFO: text -> Wv -> Wo -> x) ----
            nc.sync.dma_start(out=textN[:],
                              in_=tN_d.ap().rearrange("(g p) d -> p g d", p=128))
            nc.sync.dma_start(out=textT[:],
                              in_=tT_d.ap().rearrange("(j p) l -> p j l", p=128).bitcast(MMDT))
            nc.sync.dma_start(out=wct[:],
                              in_=wct_d.ap().rearrange("(j p) c -> p j c", p=128).bitcast(MMDT))
            nc.sync.dma_start(out=blend_sb[:], in_=bl_d.ap())
            nc.sync.dma_start(out=sg_sb[:], in_=sg_d.ap())
            nc.sync.dma_start(out=nw_sb[:], in_=nw_d.ap())

            # ---- constants built on gpsimd (idle engine) ----
            masks.make_identity(nc, ident[:])
            nc.gpsimd.memset(sel0[:], 0.0)
            nc.gpsimd.memset(sel0[0:1, :], 1.0)
            # sel1 = 1 - sel0 (gpsimd can't memset at partition offset 1)
            nc.vector.tensor_scalar(sel1[:], sel0[:], -1.0, 1.0,
                                    op0=mybir.AluOpType.mult,
                                    op1=mybir.AluOpType.add)
            nc.gpsimd.memset(cwbd[:], 0.0)
            nc.gpsimd.memset(crT_sb[:].bitcast(F32), 0.0)
            nc.gpsimd.memset(yT_sb[:].bitcast(F32), 0.0)
            nc.gpsimd.memset(eps_t[:], 1e-6)
            # m12 = [1]*C + [0]*C column; m12c its complement (no partition-
            # offset writes allowed on SBUF, so columns are built via masks)
            nc.gpsimd.memset(m12[:], 0.0)
            nc.gpsimd.memset(m12[0:C, 0:1], 1.0)
            nc.vector.tensor_scalar(m12c[:], m12[:], -1.0, 1.0,
                                    op0=mybir.AluOpType.mult,
                                    op1=mybir.AluOpType.add)
            nc.vector.tensor_copy(ones2[:, 0:1], m12[:])
            nc.vector.tensor_copy(ones2[:, 1:2], m12c[:])
            nc.vector.tensor_copy(bd4[:, 2:3], m12[:])
            nc.vector.tensor_copy(bd4[:, 3:4], m12c[:])

            # ---- logits.T = WcT.T @ textT : (C, B*L), contraction over d ----
            with tc.tile_pool(name="ps_lg", bufs=1, space="PSUM") as ps_lg:
                lg_ps = ps_lg.tile([C, BL], F32)
                for j in range(KT):
                    nc.tensor.matmul(lg_ps[:], _mm(wct[:, j, :]),
                                     _mm(textT[:, j, :]),
                                     start=(j == 0), stop=(j == KT - 1))
                nc.vector.tensor_copy(lg_sb[:], lg_ps[:])

            # ---- softmax over C per token; write into block-diag cwbd ----
            with tc.tile_pool(name="ps_lt", bufs=2, space="PSUM") as ps_lt:
                for t in range(4):
                    lt_ps = ps_lt.tile([128, C], F32)
                    nc.tensor.transpose(lt_ps[:], lg_sb[:, 128 * t:128 * (t + 1)],
                                        ident[0:C, 0:C])
                    nmx = pwrk.tile([128, 1], F32)
                    nc.vector.reduce_max(nmx[:], lt_ps[:], axis=AX, negate=True)
                    e_sb = pwrk.tile([128, C], F32)
                    nc.scalar.activation(e_sb[:], lt_ps[:], AF.Exp, bias=nmx[:])
                    ssum = pwrk.tile([128, 1], F32)
                    nc.vector.reduce_sum(ssum[:], e_sb[:], axis=AX)
                    srcp = pwrk.tile([128, 1], F32)
                    nc.vector.reciprocal(srcp[:], ssum[:])
                    off = 0 if t < 2 else C
                    nc.vector.tensor_scalar_mul(cwbd[:, t, off:off + C],
                                                e_sb[:], srcp[:])

            # ---- cr[b*c, e] = cwbd.T @ textN (contraction over l), then
            # PE-transpose the 12 rows into k-tile lhsT layout crT[d, b*c] ----
            with tc.tile_pool(name="ps_cr", bufs=1, space="PSUM") as ps_cr:
                cr_ps = [ps_cr.tile([64, 512], F32, name=f"cr{ch}",
                                    tag=f"cr{ch}") for ch in range(NCH)]
                for t in range(4):
                    for ch in range(NCH):
                        nc.tensor.matmul(cr_ps[ch][:], cwbd[:, t, :],
                                         textN[:, t, 512 * ch:512 * (ch + 1)],
                                         start=(t == 0), stop=(t == 3))
                for ch in range(NCH):
                    nc.vector.tensor_copy(cr_sb[:, 512 * ch:512 * (ch + 1)],
                                          cr_ps[ch][0:2 * C, :])
            with tc.tile_pool(name="ps_crt", bufs=2, space="PSUM") as ps_crt:
                for j in range(KT):
                    crt_ps = ps_crt.tile([128, 2 * C], F32)
                    nc.tensor.transpose(crt_ps[:],
                                        cr_sb[:, 128 * j:128 * (j + 1)],
                                        ident[0:2 * C, 0:2 * C])
                    nc.vector.tensor_copy(crT_sb[:, j, 0:2 * C], crt_ps[:])

            ptext_cm.__exit__(None, None, None)

            # ---- v[b*c, e] = crT.T @ WvT : stream WvT k-tiles from HBM ----
            pwv_cm = tc.tile_pool(name="pwv", bufs=3)
            pwv = pwv_cm.__enter__()
            with tc.tile_pool(name="ps_v", bufs=1, space="PSUM") as ps_v:
                v_ps = [ps_v.tile([64, 512], F32, name=f"v{ch}", tag=f"v{ch}")
                        for ch in range(NCH)]
                for j in range(KT):
                    wvt = pwv.tile([128, D], MMDT)
                    nc.sync.dma_start(out=wvt[:],
                                      in_=wvt_d[128 * j:128 * (j + 1), :].bitcast(MMDT))
                    for ch in range(NCH):
                        nc.tensor.matmul(v_ps[ch][:], _mm(crT_sb[:, j, :]),
                                         _mm(wvt[:, 512 * ch:512 * (ch + 1)]),
                                         start=(j == 0), stop=(j == KT - 1))
                for ch in range(NCH):
                    nc.vector.tensor_copy(v_sb[:, 512 * ch:512 * (ch + 1)],
                                          v_ps[ch][0:2 * C, :])
            pwv_cm.__exit__(None, None, None)

            # ---- x loads issued now, between the Wv and Wo streams on the
            # sync ring (bufs=8 holds the whole shard -> no slot-waits, so
            # the later Wo DMAs are never stuck behind a stalled ring) ----
            px_cm = tc.tile_pool(name="px", bufs=8)
            px = px_cm.__enter__()
            xts = []
            for t in range(ROWS // 128):
                xt = px.tile([128, D], F32, name=f"xt{t}", tag="xt")
                nc.sync.dma_start(out=xt[:], in_=x_d[128 * t:128 * (t + 1), :])
                xts.append(xt)

            # ---- blend softmax (tiny) + blendT into bd4 columns ----
            nc.vector.reduce_max(bmx[:], blend_sb[:], axis=AX, negate=True)
            nc.scalar.activation(blendn[:], blend_sb[:], AF.Exp, bias=bmx[:])
            nc.vector.reduce_sum(bsum[:], blendn[:], axis=AX)
            nc.vector.reciprocal(brcp[:], bsum[:])
            nc.vector.tensor_scalar_mul(blendn[:], blendn[:], brcp[:])
            nc.vector.tensor_copy(blendn2[0:1, 0:C], blendn[:])
            nc.vector.tensor_copy(blendn2[0:1, C:2 * C], blendn[:])
            with tc.tile_pool(name="ps_bl", bufs=1, space="PSUM") as ps_bl:
                blt_ps = ps_bl.tile([2 * C, 1], F32)
                nc.tensor.transpose(blt_ps[:], blendn2[:], ident[0:1, 0:1])
                nc.vector.tensor_mul(bd4[:, 0:1], blt_ps[:], m12[:])
                nc.vector.tensor_mul(bd4[:, 1:2], blt_ps[:], m12c[:])

            if is_surreal:
                nc.vector.tensor_mul(v2_sb[:], v_sb[:], v_sb[:])
                # g3 = sigmoid(gate) * 0.3 / (C - 1)
                nc.scalar.activation(g3[:], sg_sb[:], AF.Sigmoid)
                nc.scalar.mul(g3[:], g3[:], 0.3 / (C - 1))

            # ---- fused[b, e] = blend@v + g3*(s2 - s1^2/C) ----
            with (
                tc.tile_pool(name="ps_fl", bufs=2, space="PSUM") as ps_fl,
                tc.tile_pool(name="ps_s2", bufs=2, space="PSUM") as ps_s2,
            ):
                for ch in range(NCH):
                    sl = slice(512 * ch, 512 * (ch + 1))
                    fl_ps = ps_fl.tile([2, 512], F32)
                    nc.tensor.matmul(fl_ps[:], bd4[:, 0:2], v_sb[:, sl],
                                     start=True, stop=True)
                    if is_surreal:
                        s1_ps = ps_fl.tile([2, 512], F32, name=f"s1_{ch}",
                                           tag="s1")
                        nc.tensor.matmul(s1_ps[:], bd4[:, 2:4], v_sb[:, sl],
                                         start=True, stop=True)
                        s2_ps = ps_s2.tile([2, 512], F32)
                        nc.tensor.matmul(s2_ps[:], ones2[:], v2_sb[:, sl],
                                         start=True, stop=True)
                        t1 = pwrk.tile([2, 512], F32)
                        nc.scalar.activation(t1[:], s1_ps[:], AF.Square)
                        t2 = pwrk.tile([2, 512], F32)
                        nc.vector.scalar_tensor_tensor(
                            t2[:], t1[:], -1.0 / C, s2_ps[:],
                            op0=mybir.AluOpType.mult, op1=mybir.AluOpType.add)
                        nc.vector.scalar_tensor_tensor(
                            fused_sb[:, sl], t2[:], g3[0:2, 0:1], fl_ps[:],
                            op0=mybir.AluOpType.mult, op1=mybir.AluOpType.add)
                    else:
                        nc.vector.tensor_copy(fused_sb[:, sl], fl_ps[:])

            # ---- rmsnorm: y = fused * rsqrt(mean(fused^2) + eps) * nw ----
            nc.vector.tensor_mul(sq_sb[:], fused_sb[:], fused_sb[:])
            nc.vector.reduce_sum(ms[:], sq_sb[:], axis=AX)
            nc.scalar.activation(ms[:], ms[:], AF.Sqrt, bias=eps_t[:],
                                 scale=1.0 / D)
            nc.vector.reciprocal(rs[:], ms[:])
            nc.vector.scalar_tensor_tensor(
                y_sb[:], fused_sb[:], rs[0:2, 0:1], nw_sb[:],
                op0=mybir.AluOpType.mult, op1=mybir.AluOpType.mult)

            # ---- yT tiles via PE transpose ----
            with tc.tile_pool(name="ps_yt", bufs=2, space="PSUM") as ps_yt:
                for j in range(KT):
                    yt_ps = ps_yt.tile([128, 2], F32)
                    nc.tensor.transpose(yt_ps[:], y_sb[:, 128 * j:128 * (j + 1)],
                                        ident[0:2, 0:2])
                    nc.vector.tensor_copy(yT_sb[:, j, 0:2], yt_ps[:])

                # ---- obd[b, e'] = yT.T @ WoT : stream WoT k-tiles ----
                with (
                    tc.tile_pool(name="pwo", bufs=4) as pwo,
                    tc.tile_pool(name="ps_ob", bufs=1, space="PSUM") as ps_ob,
                ):
                    ob_ps = [ps_ob.tile([64, 512], F32, name=f"ob{ch}", tag=f"ob{ch}")
                             for ch in range(NCH)]
                    for j in range(KT):
                        wot = pwo.tile([128, D], MMDT)
                        nc.sync.dma_start(out=wot[:],
                                          in_=wot_d[128 * j:128 * (j + 1), :].bitcast(MMDT))
                        for ch in range(NCH):
                            nc.tensor.matmul(ob_ps[ch][:], _mm(yT_sb[:, j, :]),
                                             _mm(wot[:, 512 * ch:512 * (ch + 1)]),
                                             start=(j == 0), stop=(j == KT - 1))
                    for ch in range(NCH):
                        nc.vector.tensor_copy(obd_sb[:, 512 * ch:512 * (ch + 1)],
                                              ob_ps[ch][0:2, :])

            # ---- broadcast obd rows to 128 partitions via k=1 matmul ----
            with tc.tile_pool(name="ps_bc", bufs=2, space="PSUM") as ps_bc:
                for sel, bc in ((sel0, bc0), (sel1, bc1)):
                    for ch in range(NCH):
                        sl = slice(512 * ch, 512 * (ch + 1))
                        bc_ps = ps_bc.tile([128, 512], F32)
                        nc.tensor.matmul(bc_ps[:], sel[:], obd_sb[0:2, sl],
                                         start=True, stop=True)
                        nc.vector.tensor_copy(bc[:, sl], bc_ps[:])

            # ---- the only O(N) work: out = x + obd[b] (rows 0..511 are b0) ----
            for t in range(ROWS // 128):
                xt = xts[t]
                bc = bc0 if t < (HALF // 128) else bc1
                eng = nc.gpsimd if t % 4 == 3 else nc.vector
                eng.tensor_add(xt[:], xt[:], bc[:])
                nc.scalar.dma_start(out=out_d[128 * t:128 * (t + 1), :],
                                    in_=xt[:])
            px_cm.__exit__(None, None, None)

    nc.compile()
    return nc


def prep_inputs(x, text_emb, Wc, Wv, Wo, blend_weights, surreal_gate,
                norm_weight):
    """Host-side layout prep (slice/transpose/replicate only)."""
    f = np.float32
    shared = {
        "textN": np.ascontiguousarray(text_emb.reshape(BL, D), dtype=f),
        "textT": np.ascontiguousarray(text_emb.reshape(BL, D).T, dtype=f),
        "WcT": np.ascontiguousarray(Wc.T, dtype=f),
        "WvT": np.ascontiguousarray(Wv.T, dtype=f),
        "WoT": np.ascontiguousarray(Wo.T, dtype=f),
        "blend": np.ascontiguousarray(blend_weights.reshape(1, C), dtype=f),
        "sg2": np.broadcast_to(np.asarray(surreal_gate, f).reshape(1, 1),
                               (2, 1)).copy(),
        "nw2": np.broadcast_to(np.asarray(norm_weight, f), (2, D)).copy(),
    }
    in_maps = []
    for k in range(N_CORES):
        xs = np.concatenate(
            [x[0, HALF * k:HALF * (k + 1), :], x[1, HALF * k:HALF * (k + 1), :]],
            axis=0).astype(f)
        in_maps.append({"x_shard": np.ascontiguousarray(xs), **shared})
    return in_maps


_CACHE = {}


def kernel(x, text_emb, Wc, Wq, Wk, Wv, Wo, blend_weights, surreal_gate,
           norm_weight, is_surreal, _collect=None):
    surreal = bool(int(np.asarray(is_surreal)))
    key = ("nc", surreal)
    if key not in _CACHE:
        _CACHE[key] = build_nc(surreal)
    nc = _CACHE[key]

    in_maps = prep_inputs(x, text_emb, Wc, Wv, Wo, blend_weights,
                          surreal_gate, norm_weight)
    res = run_bass_kernel_spmd(
        nc, in_maps, core_ids=list(range(N_CORES)),
        trace=os.environ.get("KERNEL_TRACE", "0") == "1",
    )
    if _collect is not None:
        _collect.append(res)

    out = np.empty((B, N, D), np.float32)
    for k in range(N_CORES):
        shard = res.results[k]["out_shard"]
        out[0, HALF * k:HALF * (k + 1), :] = shard[:HALF]
        out[1, HALF * k:HALF * (k + 1), :] = shard[HALF:]
    return out



# revision 11
# speedup vs baseline: 1.4395x; 1.4395x over previous
"""Trainium2 Bass kernel for nn_ConceptFusionModule (8-core, collective).

Math: softmax over a single key collapses the SDPA, so the module reduces to

    cw      = softmax(textN @ Wc.T, -1)           # (B*L, C)
    cr      = cw.T @ textN  (per batch)           # (B, C, D)
    v       = cr @ Wv.T                           # (B, C, D)
    fused   = blend@v + sig(g)*.3*var_c(v)        # (B, D)
    obd     = rmsnorm(fused)*nw @ Wo.T            # (B, D)
    out     = x + obd[:, None, :]                 # broadcast over N

Sharding (the point of this version): the baseline replicated the whole
small chain, so every core streamed the full DxD Wv and Wo (33.5 MB of
weights per core).  Here the chain is tensor-parallel over D:

  - text rows:  64 per core -> partial crT (d-on-partitions, 6 cols)
                -> AllGather(49 KB) -> local sum (ranks 0-3 = batch 0).
  - Wv: core k holds WvT[:, 256k:256k+256]  -> v/fused for its e-slice.
  - Wo: core k holds WoT[256k:256k+256, :]  -> partial z = (fused*nw)@WoT
    (z is linear in y, so the global rmsnorm scale can be applied after
    the reduce), plus partial sum(fused^2).
  - AllReduce(16.4 KB) of [z | ssq] -> obd = z * rsqrt(ssq/D + eps).

Per-core HBM traffic: x 8.4 + out 8.4 + text 1.05 + Wv/8 2.1 + Wo/8 2.1
+ collective bounces ~1 = ~23 MB (vs 58.8 MB for the baseline).
The only O(B*N*D) work is the final broadcast add (DVE/gpsimd), fully
overlapped with the x DMA stream.
"""

import os

import numpy as np

import concourse.bacc as bacc
import concourse.bass as bass
import concourse.mybir as mybir
import concourse.tile as tile
from concourse import masks
from concourse.bass_utils import run_bass_kernel_spmd

F32 = mybir.dt.float32
F32R = mybir.dt.float32r

N_CORES = 8
B, N, L, D, C = 2, 4096, 256, 2048, 6
BL = B * L               # 512 text rows
LS = BL // N_CORES       # 64 text rows per core
ES = D // N_CORES        # 256-wide d/e slice per core
ROWS = B * N // N_CORES  # 1024 x rows per core (512 per batch)
HALF = ROWS // 2
KT = D // 128            # 16 contraction k-tiles
JW = ES // 128           # 2 k-tiles for the wot contraction
G = 2                    # 128-row tiles per x supertile
NST = ROWS // (128 * G)  # 4 supertiles
AX = mybir.AxisListType.X
AF = mybir.ActivationFunctionType
ADD = mybir.AluOpType.add
MUL = mybir.AluOpType.mult
RG = [list(range(N_CORES))]


def build_nc(is_surreal: bool) -> bacc.Bacc:
    nc = bacc.Bacc("TRN2", target_bir_lowering=False, debug=False,
                   num_devices=N_CORES)

    x_d = nc.dram_tensor("x_shard", [ROWS, D], F32, kind="ExternalInput")
    tN_d = nc.dram_tensor("tN", [LS, D], F32, kind="ExternalInput")
    tT_d = nc.dram_tensor("tT", [D, LS], F32, kind="ExternalInput")
    wct_d = nc.dram_tensor("WcT", [D, C], F32, kind="ExternalInput")
    wvt_d = nc.dram_tensor("wvt", [D, ES], F32, kind="ExternalInput")
    wot_d = nc.dram_tensor("wot", [ES, D], F32, kind="ExternalInput")
    bl_d = nc.dram_tensor("blend", [1, C], F32, kind="ExternalInput")
    sg_d = nc.dram_tensor("sg", [1, 1], F32, kind="ExternalInput")
    nw_d = nc.dram_tensor("nw2", [1, 2 * ES], F32, kind="ExternalInput")
    out_d = nc.dram_tensor("out_shard", [ROWS, D], F32, kind="ExternalOutput")

    with tile.TileContext(nc) as tc:
        with (
            tc.tile_pool(name="pc", bufs=1) as pc,
            tc.tile_pool(name="pwrk", bufs=2) as pwrk,
            tc.tile_pool(name="dram", bufs=1, space="DRAM") as dram,
        ):
            # ---- persistent SBUF tiles ----
            tT_sb = pc.tile([128, KT, LS], F32R)
            tN_sb = pc.tile([LS, D], F32R)
            wct_sb = pc.tile([128, KT, C], F32R)
            wvt_sb = pc.tile([128, KT, ES], F32R)
            wot_sb = pc.tile([128, JW, D], F32R)
            bl_sb = pc.tile([1, C], F32)
            sg_sb = pc.tile([1, 1], F32)
            nw_sb = pc.tile([1, 2 * ES], F32)
            ident = pc.tile([128, 128], F32)
            eps_t = pc.tile([2, 1], F32)
            ones6 = pc.tile([C, 1], F32R)
            blendn = pc.tile([1, C], F32)
            blT = pc.tile([C, 1], F32R)
            g3 = pc.tile([1, 1], F32)
            lg_sb = pc.tile([LS, C], F32)
            cwr = pc.tile([LS, C], F32R)
            crtp_sb = pc.tile([128, KT, C], F32)
            agl_sb = pc.tile([128, N_CORES, KT * C], F32)
            crTw = pc.tile([128, KT * C], F32)
            crTb0 = pc.tile([128, KT * C], F32R)
            crTb1 = pc.tile([128, KT * C], F32R)
            v_sb = pc.tile([C, 2 * ES], F32R)
            v2_sb = pc.tile([C, 2 * ES], F32R)
            fused = pc.tile([1, 2 * ES], F32)
            fy_sb = pc.tile([1, 2 * ES], F32)
            sq3 = pc.tile([1, 2, ES], F32)
            ssq12 = pc.tile([1, 2], F32)
            fyT = pc.tile([128, JW, 2], F32R)
            z_sb = pc.tile([2, D + 1], F32R)
            zr_sb = pc.tile([2, D + 1], F32R)
            ms = pc.tile([2, 1], F32)
            rs = pc.tile([2, 1], F32)
            sel0 = pc.tile([2, 128], F32)
            sel1 = pc.tile([2, 128], F32)
            selw = pc.tile([2, 128], F32)
            sel0s = pc.tile([2, 128], F32R)
            sel1s = pc.tile([2, 128], F32R)
            bc0 = pc.tile([128, D], F32)
            bc1 = pc.tile([128, D], F32)

            # ---- internal DRAM (collective bounce buffers) ----
            ag1_in = dram.tile([128, KT * C], F32)
            ag1_out = dram.tile([128 * N_CORES, KT * C], F32)
            ag2_in = dram.tile([2, D + 1], F32)
            ag2_out = dram.tile([2, D + 1], F32)

            # ---- all loads on the sync HWDGE ring, critical-path first ----
            nc.sync.dma_start(out=tT_sb[:],
                              in_=tT_d.ap().rearrange("(j p) l -> p j l",
                                                      p=128).bitcast(F32R))
            nc.sync.dma_start(out=tN_sb[:], in_=tN_d.ap().bitcast(F32R))
            nc.sync.dma_start(out=wct_sb[:],
                              in_=wct_d.ap().rearrange("(j p) c -> p j c",
                                                       p=128).bitcast(F32R))
            nc.sync.dma_start(out=bl_sb[:], in_=bl_d.ap())
            nc.sync.dma_start(out=sg_sb[:], in_=sg_d.ap())
            nc.sync.dma_start(out=nw_sb[:], in_=nw_d.ap())
            nc.sync.dma_start(out=wvt_sb[:],
                              in_=wvt_d.ap().rearrange("(j p) e -> p j e",
                                                       p=128).bitcast(F32R))
            nc.sync.dma_start(out=wot_sb[:],
                              in_=wot_d.ap().rearrange("(j p) e -> p j e",
                                                       p=128).bitcast(F32R))
            px_cm = tc.tile_pool(name="px", bufs=1)
            px = px_cm.__enter__()
            xts = []
            for s in range(NST):
                xt = px.tile([128, G, D], F32, name=f"xst{s}")
                nc.sync.dma_start(
                    out=xt[:],
                    in_=x_d[128 * G * s:128 * G * (s + 1), :].rearrange(
                        "(g p) d -> p g d", p=128))
                xts.append(xt)

            # ---- constants ----
            masks.make_identity(nc, ident[:])
            nc.gpsimd.memset(eps_t[:], 1e-6)
            nc.gpsimd.memset(ones6[:].bitcast(F32), 1.0)
            nc.gpsimd.memset(sel0[:], 0.0)
            nc.gpsimd.memset(sel0[0:1, :], 1.0)
            nc.vector.tensor_scalar(sel1[:], sel0[:], -1.0, 1.0,
                                    op0=MUL, op1=ADD)

            # ---- blend softmax + g3 (tiny, independent -> early) ----
            bmx = pwrk.tile([1, 1], F32)
            nc.vector.reduce_max(bmx[:], bl_sb[:], axis=AX, negate=True)
            nc.scalar.activation(blendn[:], bl_sb[:], AF.Exp, bias=bmx[:])
            bsum = pwrk.tile([1, 1], F32)
            nc.vector.reduce_sum(bsum[:], blendn[:], axis=AX)
            brcp = pwrk.tile([1, 1], F32)
            nc.vector.reciprocal(brcp[:], bsum[:])
            nc.vector.tensor_scalar_mul(blendn[:], blendn[:], brcp[:])
            if is_surreal:
                nc.scalar.activation(g3[:], sg_sb[:], AF.Sigmoid)
                nc.scalar.mul(g3[:], g3[:], 0.3 / (C - 1))

            # ---- logits (LS, C) = tT.T @ WcT, contraction over d ----
            with tc.tile_pool(name="ps_lg", bufs=1, space="PSUM") as ps_lg:
                lg_ps = ps_lg.tile([LS, C], F32)
                for j in range(KT):
                    nc.tensor.matmul(lg_ps[:], tT_sb[:, j, :], wct_sb[:, j, :],
                                     start=(j == 0), stop=(j == KT - 1))
                nc.vector.tensor_copy(lg_sb[:], lg_ps[:])

            # ---- softmax over C per token ----
            nmx = pwrk.tile([LS, 1], F32)
            nc.vector.reduce_max(nmx[:], lg_sb[:], axis=AX, negate=True)
            e_sb = pwrk.tile([LS, C], F32)
            nc.scalar.activation(e_sb[:], lg_sb[:], AF.Exp, bias=nmx[:])
            ssum = pwrk.tile([LS, 1], F32)
            nc.vector.reduce_sum(ssum[:], e_sb[:], axis=AX)
            srcp = pwrk.tile([LS, 1], F32)
            nc.vector.reciprocal(srcp[:], ssum[:])
            cw_sb = pwrk.tile([LS, C], F32)
            nc.vector.tensor_scalar_mul(cw_sb[:], e_sb[:], srcp[:])
            nc.vector.tensor_copy(cwr[:], cw_sb[:])

            # ---- partial crT[d, c] = textN.T @ cw (contraction over l) ----
            with tc.tile_pool(name="ps_cp", bufs=1, space="PSUM") as ps_cp:
                crtp_ps = ps_cp.tile([128, KT, C], F32)
                for j in range(KT):
                    nc.tensor.matmul(crtp_ps[:, j, :],
                                     tN_sb[:, 128 * j:128 * (j + 1)], cwr[:],
                                     start=True, stop=True)
                nc.vector.tensor_copy(crtp_sb[:], crtp_ps[:])

            # ---- AllGather partial crT; ranks 0-3 are batch 0 ----
            nc.scalar.dma_start(
                out=ag1_in[:].rearrange("p (j c) -> p j c", j=KT),
                in_=crtp_sb[:])
            nc.gpsimd.collective_compute(
                "AllGather", mybir.AluOpType.bypass, replica_groups=RG,
                ins=[ag1_in.opt()], outs=[ag1_out.opt()])
            nc.scalar.dma_start(
                out=agl_sb[:],
                in_=ag1_out[:].rearrange("(r p) f -> p r f", p=128))
            for half, dst in ((0, crTb0), (1, crTb1)):
                nc.vector.tensor_copy(crTw[:], agl_sb[:, 4 * half, :])
                for r in range(1, 4):
                    nc.vector.tensor_add(crTw[:], crTw[:],
                                         agl_sb[:, 4 * half + r, :])
                nc.vector.tensor_copy(dst[:], crTw[:])

            # ---- blendT column (needs PE; placed between matmul phases) ----
            with tc.tile_pool(name="ps_bt", bufs=1, space="PSUM") as ps_bt:
                blt_ps = ps_bt.tile([C, 1], F32)
                nc.tensor.transpose(blt_ps[:], blendn[:], ident[0:1, 0:1])
                nc.vector.tensor_copy(blT[:], blt_ps[:])

            if True:
                # ---- v[c, e] for both batches: [v_b0 | v_b1] cols ----
                with tc.tile_pool(name="ps_v", bufs=1, space="PSUM") as ps_v:
                    v_ps0 = ps_v.tile([C, ES], F32)
                    v_ps1 = ps_v.tile([C, ES], F32)
                    for j in range(KT):
                        nc.tensor.matmul(v_ps0[:],
                                         crTb0[:, C * j:C * (j + 1)],
                                         wvt_sb[:, j, :],
                                         start=(j == 0), stop=(j == KT - 1))
                        nc.tensor.matmul(v_ps1[:],
                                         crTb1[:, C * j:C * (j + 1)],
                                         wvt_sb[:, j, :],
                                         start=(j == 0), stop=(j == KT - 1))
                    nc.vector.tensor_copy(v_sb[:, 0:ES], v_ps0[:])
                    nc.vector.tensor_copy(v_sb[:, ES:2 * ES], v_ps1[:])
                    if is_surreal:
                        nc.vector.tensor_mul(v2_sb[:, 0:ES], v_sb[:, 0:ES],
                                             v_ps0[:])
                        nc.vector.tensor_mul(v2_sb[:, ES:2 * ES],
                                             v_sb[:, ES:2 * ES], v_ps1[:])

                # ---- fused[1, 2*ES] = blend@v + g3*(s2 - s1^2/C) ----
                with tc.tile_pool(name="ps_d", bufs=1, space="PSUM") as ps_d:
                    fl_ps = ps_d.tile([1, 2 * ES], F32)
                    nc.tensor.matmul(fl_ps[:], blT[:], v_sb[:],
                                     start=True, stop=True)
                    if is_surreal:
                        s1_ps = ps_d.tile([1, 2 * ES], F32)
                        nc.tensor.matmul(s1_ps[:], ones6[:], v_sb[:],
                                         start=True, stop=True)
                        s2_ps = ps_d.tile([1, 2 * ES], F32)
                        nc.tensor.matmul(s2_ps[:], ones6[:], v2_sb[:],
                                         start=True, stop=True)
                        t1 = pwrk.tile([1, 2 * ES], F32)
                        nc.scalar.activation(t1[:], s1_ps[:], AF.Square)
                        t2 = pwrk.tile([1, 2 * ES], F32)
                        nc.vector.scalar_tensor_tensor(
                            t2[:], t1[:], -1.0 / C, s2_ps[:],
                            op0=MUL, op1=ADD)
                        nc.vector.scalar_tensor_tensor(
                            fused[:], t2[:], g3[0:1, 0:1], fl_ps[:],
                            op0=MUL, op1=ADD)
                    else:
                        nc.vector.tensor_copy(fused[:], fl_ps[:])

                # ---- fy = fused * nw; partial ssq per batch ----
                nc.vector.tensor_mul(fy_sb[:], fused[:], nw_sb[:])
                nc.vector.tensor_mul(sq3[:, 0, :], fused[:, 0:ES],
                                     fused[:, 0:ES])
                nc.vector.tensor_mul(sq3[:, 1, :], fused[:, ES:2 * ES],
                                     fused[:, ES:2 * ES])
                nc.vector.reduce_sum(ssq12[:], sq3[:], axis=AX)

                # ---- fyT[d-tile, b] via PE transpose ----
                with tc.tile_pool(name="ps_tr", bufs=2, space="PSUM") as ps_tr:
                    for ci in range(2 * JW):
                        j, b = ci % JW, ci // JW
                        tp = ps_tr.tile([128, 1], F32, name=f"tp{ci}",
                                        tag="tp")
                        nc.tensor.transpose(tp[:],
                                            fy_sb[:, 128 * ci:128 * (ci + 1)],
                                            ident[0:1, 0:1])
                        nc.vector.tensor_copy(fyT[:, j, b:b + 1], tp[:])
                    tq = ps_tr.tile([2, 1], F32)
                    nc.tensor.transpose(tq[:], ssq12[:], ident[0:1, 0:1])
                    nc.vector.tensor_copy(z_sb[:, D:D + 1], tq[:])

            # ---- partial z[b, e'] = fyT.T @ WoT rows (d-slice) ----
            with tc.tile_pool(name="ps_z", bufs=1, space="PSUM") as ps_z:
                zps = [ps_z.tile([2, 512], F32, name=f"zp{ch}", tag=f"zp{ch}")
                       for ch in range(D // 512)]
                for j in range(JW):
                    for ch in range(D // 512):
                        nc.tensor.matmul(zps[ch][:],
                                         fyT[:, j, :],
                                         wot_sb[:, j, 512 * ch:512 * (ch + 1)],
                                         start=(j == 0), stop=(j == JW - 1))
                for ch in range(D // 512):
                    nc.vector.tensor_copy(z_sb[:, 512 * ch:512 * (ch + 1)],
                                          zps[ch][:])

            # ---- AllReduce [z | ssq]; then obd = z * rsqrt(ssq/D + eps) ----
            nc.scalar.dma_start(out=ag2_in[:], in_=z_sb[:].bitcast(F32))
            nc.gpsimd.collective_compute(
                "AllReduce", ADD, replica_groups=RG,
                ins=[ag2_in.opt()], outs=[ag2_out.opt()])
            nc.scalar.dma_start(out=zr_sb[:], in_=ag2_out[:].bitcast(F32R))
            nc.scalar.activation(ms[:], zr_sb[:, D:D + 1].bitcast(F32),
                                 AF.Sqrt, bias=eps_t[:], scale=1.0 / D)
            nc.vector.reciprocal(rs[:], ms[:])
            nc.vector.tensor_scalar_mul(selw[:], sel0[:], rs[:])
            nc.vector.tensor_copy(sel0s[:], selw[:])
            nc.vector.tensor_scalar_mul(selw[:], sel1[:], rs[:])
            nc.vector.tensor_copy(sel1s[:], selw[:])
            with tc.tile_pool(name="ps_bc", bufs=2, space="PSUM") as ps_bc:
                for sel, bc in ((sel0s, bc0), (sel1s, bc1)):
                    for ch in range(D // 512):
                        bc_ps = ps_bc.tile([128, 512], F32, name="bc_ps",
                                           tag="bc_ps")
                        nc.tensor.matmul(bc_ps[:], sel[:],
                                         zr_sb[:, 512 * ch:512 * (ch + 1)],
                                         start=True, stop=True)
                        nc.vector.tensor_copy(bc[:, 512 * ch:512 * (ch + 1)],
                                              bc_ps[:])

            # ---- the only O(N) work: out = x + obd[b] ----
            for s in range(NST):
                bc = bc0 if s < NST // 2 else bc1
                for g in range(G):
                    t_idx = s * G + g
                    eng = nc.gpsimd if t_idx % 4 == 3 else nc.vector
                    eng.tensor_add(xts[s][:, g, :], xts[s][:, g, :], bc[:])
                nc.scalar.dma_start(
                    out=out_d[128 * G * s:128 * G * (s + 1), :].rearrange(
                        "(g p) d -> p g d", p=128),
                    in_=xts[s][:])
            px_cm.__exit__(None, None, None)

    nc.compile()
    return nc


def prep_inputs(x, text_emb, Wc, Wv, Wo, blend_weights, surreal_gate,
                norm_weight):
    """Host-side layout prep (slice/transpose/replicate only)."""
    f = np.float32
    textN = np.asarray(text_emb, f).reshape(BL, D)
    WcT = np.ascontiguousarray(np.asarray(Wc, f).T)
    WvT = np.ascontiguousarray(np.asarray(Wv, f).T)
    WoT = np.ascontiguousarray(np.asarray(Wo, f).T)
    nw = np.asarray(norm_weight, f)
    blend = np.ascontiguousarray(np.asarray(blend_weights, f).reshape(1, C))
    sg = np.asarray(surreal_gate, f).reshape(1, 1)
    in_maps = []
    for k in range(N_CORES):
        ls = slice(LS * k, LS * (k + 1))
        es = slice(ES * k, ES * (k + 1))
        tN = np.ascontiguousarray(textN[ls])
        xs = np.concatenate(
            [x[0, HALF * k:HALF * (k + 1), :], x[1, HALF * k:HALF * (k + 1), :]],
            axis=0).astype(f)
        in_maps.append({
            "x_shard": np.ascontiguousarray(xs),
            "tN": tN,
            "tT": np.ascontiguousarray(tN.T),
            "WcT": WcT,
            "wvt": np.ascontiguousarray(WvT[:, es]),
            "wot": np.ascontiguousarray(WoT[es, :]),
            "blend": blend,
            "sg": sg,
            "nw2": np.concatenate([nw[es], nw[es]])[None, :].astype(f),
        })
    return in_maps


_CACHE = {}


def kernel(x, text_emb, Wc, Wq, Wk, Wv, Wo, blend_weights, surreal_gate,
           norm_weight, is_surreal, _collect=None):
    surreal = bool(int(np.asarray(is_surreal)))
    key = ("nc", surreal)
    if key not in _CACHE:
        _CACHE[key] = build_nc(surreal)
    nc = _CACHE[key]

    in_maps = prep_inputs(x, text_emb, Wc, Wv, Wo, blend_weights,
                          surreal_gate, norm_weight)
    res = run_bass_kernel_spmd(
        nc, in_maps, core_ids=list(range(N_CORES)),
        trace=os.environ.get("KERNEL_TRACE", "0") == "1",
    )
    if _collect is not None:
        _collect.append(res)

    out = np.empty((B, N, D), np.float32)
    for k in range(N_CORES):
        shard = res.results[k]["out_shard"]
        out[0, HALF * k:HALF * (k + 1), :] = shard[:HALF]
        out[1, HALF * k:HALF * (k + 1), :] = shard[HALF:]
    return out


# revision 13
# speedup vs baseline: 1.5162x; 1.0533x over previous
"""Trainium2 Bass kernel for nn_ConceptFusionModule (8-core, collective).

Math: softmax over a single key collapses the SDPA, so the module reduces to

    cw      = softmax(textN @ Wc.T, -1)           # (B*L, C)
    cr      = cw.T @ textN  (per batch)           # (B, C, D)
    v       = cr @ Wv.T                           # (B, C, D)
    fused   = blend@v + sig(g)*.3*var_c(v)        # (B, D)
    obd     = rmsnorm(fused)*nw @ Wo.T            # (B, D)
    out     = x + obd[:, None, :]                 # broadcast over N

Sharding: the chain is tensor-parallel over D so no core streams the full
DxD weights (the baseline's 33.5 MB/core of Wv+Wo becomes 4.2 MB/core):

  - text rows: 64 per core -> partial crT[d, b*c] with the batch block
    picked by a data-driven column mask (SPMD-uniform program)
    -> AllReduce #1 (98 KB).
  - Wv: core k holds WvT[:, 256k:256k+256]  -> v/fused for its e-slice.
  - Wo: core k holds WoT[256k:256k+256, :]  -> partial z = (fused*nw)@WoT
    (z is linear in y, so the global rmsnorm scale is applied after the
    reduce), plus partial sum(fused^2) duplicated into both payload rows.
  - AllReduce #2 (16.4 KB) of [z | ssq] -> obd = z * rsqrt(ssq/D + eps),
    with the rsqrt folded into the final add as a per-partition scalar.

A dummy 32-byte AllGather is triggered first thing so the runtime's
one-time 8-core entry barrier (which absorbs core-launch skew) overlaps
the x/weight DMA stream instead of the collective chain.

Per-core HBM traffic: x 8.4 + out 8.4 + text 1.05 + Wv/8 2.1 + Wo/8 2.1
+ bounces ~0.3 = ~22.4 MB (vs 58.8 MB for the baseline).  The only
O(B*N*D) work is the final broadcast add: DVE reads the PE-broadcast
obd rows straight from PSUM and fuses the rsqrt scale, while the three
DMA rings (scalar/sync/gpsimd) share the store stream.
"""

import os

import numpy as np

import concourse.bacc as bacc
import concourse.bass as bass
import concourse.mybir as mybir
import concourse.tile as tile
from concourse import masks
from concourse.bass_utils import run_bass_kernel_spmd

F32 = mybir.dt.float32
F32R = mybir.dt.float32r

N_CORES = 8
B, N, L, D, C = 2, 4096, 256, 2048, 6
BL = B * L               # 512 text rows
LS = BL // N_CORES       # 64 text rows per core
ES = D // N_CORES        # 256-wide d/e slice per core
ROWS = B * N // N_CORES  # 1024 x rows per core (512 per batch)
HALF = ROWS // 2
KT = D // 128            # 16 contraction k-tiles
JW = ES // 128           # 2 k-tiles for the wot contraction
G = 2                    # 128-row tiles per x supertile
NST = ROWS // (128 * G)  # 4 supertiles
AX = mybir.AxisListType.X
AF = mybir.ActivationFunctionType
ADD = mybir.AluOpType.add
MUL = mybir.AluOpType.mult
RG = [list(range(N_CORES))]


def build_nc(is_surreal: bool) -> bacc.Bacc:
    nc = bacc.Bacc("TRN2", target_bir_lowering=False, debug=False,
                   num_devices=N_CORES)

    x_d = nc.dram_tensor("x_shard", [ROWS, D], F32, kind="ExternalInput")
    tN_d = nc.dram_tensor("tN", [LS, D], F32, kind="ExternalInput")
    tT_d = nc.dram_tensor("tT", [D, LS], F32, kind="ExternalInput")
    wct_d = nc.dram_tensor("WcT", [D, C], F32, kind="ExternalInput")
    wvt_d = nc.dram_tensor("wvt", [D, ES], F32, kind="ExternalInput")
    wot_d = nc.dram_tensor("wot", [ES, D], F32, kind="ExternalInput")
    bl_d = nc.dram_tensor("blend", [1, C], F32, kind="ExternalInput")
    sg_d = nc.dram_tensor("sg", [1, 1], F32, kind="ExternalInput")
    nw_d = nc.dram_tensor("nw2", [1, 2 * ES], F32, kind="ExternalInput")
    bm_d = nc.dram_tensor("bm", [LS, 2], F32, kind="ExternalInput")
    out_d = nc.dram_tensor("out_shard", [ROWS, D], F32, kind="ExternalOutput")

    with tile.TileContext(nc) as tc:
        with (
            tc.tile_pool(name="pc", bufs=1) as pc,
            tc.tile_pool(name="pwrk", bufs=2) as pwrk,
            tc.tile_pool(name="dram", bufs=1, space="DRAM") as dram,
        ):
            # ---- persistent SBUF tiles ----
            tT_sb = pc.tile([128, KT, LS], F32R)
            tN_sb = pc.tile([LS, D], F32R)
            wct_sb = pc.tile([128, KT, C], F32R)
            wvt_sb = pc.tile([128, KT, ES], F32R)
            wot_sb = pc.tile([128, JW, D], F32R)
            bl_sb = pc.tile([1, C], F32)
            sg_sb = pc.tile([1, 1], F32)
            nw_sb = pc.tile([1, 2 * ES], F32)
            bm_sb = pc.tile([LS, 2], F32)
            ident = pc.tile([128, 128], F32)
            eps_t = pc.tile([1, 1], F32)
            ones6 = pc.tile([C, 1], F32R)
            dum_sb = pc.tile([1, 8], F32)
            blendn = pc.tile([1, C], F32)
            blT = pc.tile([C, 1], F32R)
            g3 = pc.tile([1, 1], F32)
            cwb = pc.tile([LS, 2 * C], F32R)
            crtp_sb = pc.tile([128, KT, 2 * C], F32)
            crT_sb = pc.tile([128, KT, 2 * C], F32R)
            v_sb = pc.tile([C, 2 * ES], F32R)
            v2_sb = pc.tile([C, 2 * ES], F32R)
            fused = pc.tile([1, 2 * ES], F32)
            fy_sb = pc.tile([1, 2 * ES], F32)
            sq3 = pc.tile([1, 2, ES], F32)
            ssq12 = pc.tile([1, 2], F32)
            ssq2x = pc.tile([2, 2], F32)
            fyT = pc.tile([128, JW, 2], F32R)
            z_sb = pc.tile([2, D + 2], F32R)
            zr_sb = pc.tile([2, D + 2], F32R)
            rs2 = pc.tile([1, 2], F32)
            rsB = pc.tile([128, 2], F32)
            sel0 = pc.tile([2, 128], F32)
            sel0r = pc.tile([2, 128], F32R)
            sel1r = pc.tile([2, 128], F32R)

            # ---- internal DRAM (collective bounce buffers) ----
            dum_in = dram.tile([1, 8], F32)
            dum_out = dram.tile([N_CORES, 8], F32)
            ar1_in = dram.tile([128, KT * 2 * C], F32)
            ar1_out = dram.tile([128, KT * 2 * C], F32)
            ar2_in = dram.tile([2, D + 2], F32)
            ar2_out = dram.tile([2, D + 2], F32)

            # ---- dummy collective: eats the one-time 8-core entry
            # barrier (core-launch skew) while the DMA stream runs ----
            nc.gpsimd.memset(dum_sb[:], 0.0)
            nc.gpsimd.dma_start(out=dum_in[:], in_=dum_sb[:])
            nc.gpsimd.collective_compute(
                "AllGather", mybir.AluOpType.bypass, replica_groups=RG,
                ins=[dum_in.opt()], outs=[dum_out.opt()])

            # ---- all loads on the sync HWDGE ring, critical-path first ----
            nc.sync.dma_start(out=tT_sb[:],
                              in_=tT_d.ap().rearrange("(j p) l -> p j l",
                                                      p=128).bitcast(F32R))
            nc.sync.dma_start(out=tN_sb[:], in_=tN_d.ap().bitcast(F32R))
            nc.sync.dma_start(out=wct_sb[:],
                              in_=wct_d.ap().rearrange("(j p) c -> p j c",
                                                       p=128).bitcast(F32R))
            nc.sync.dma_start(out=bl_sb[:], in_=bl_d.ap())
            nc.sync.dma_start(out=sg_sb[:], in_=sg_d.ap())
            nc.sync.dma_start(out=nw_sb[:], in_=nw_d.ap())
            nc.sync.dma_start(out=bm_sb[:], in_=bm_d.ap())
            nc.sync.dma_start(out=wvt_sb[:],
                              in_=wvt_d.ap().rearrange("(j p) e -> p j e",
                                                       p=128).bitcast(F32R))
            nc.sync.dma_start(out=wot_sb[:],
                              in_=wot_d.ap().rearrange("(j p) e -> p j e",
                                                       p=128).bitcast(F32R))
            px_cm = tc.tile_pool(name="px", bufs=1)
            px = px_cm.__enter__()
            xts = []
            for s in range(NST):
                xt = px.tile([128, G, D], F32, name=f"xst{s}")
                nc.sync.dma_start(
                    out=xt[:],
                    in_=x_d[128 * G * s:128 * G * (s + 1), :].rearrange(
                        "(g p) d -> p g d", p=128))
                xts.append(xt)

            # ---- constants ----
            masks.make_identity(nc, ident[:])
            nc.gpsimd.memset(eps_t[:], 1e-6)
            nc.gpsimd.memset(ones6[:].bitcast(F32), 1.0)
            nc.gpsimd.memset(sel0[:], 0.0)
            nc.gpsimd.memset(sel0[0:1, :], 1.0)
            nc.vector.tensor_copy(sel0r[:], sel0[:])
            nc.vector.tensor_scalar(sel1r[:], sel0[:], -1.0, 1.0,
                                    op0=MUL, op1=ADD)

            # ---- blend softmax + g3 (tiny, independent -> early) ----
            nc.scalar.activation(blendn[:], bl_sb[:], AF.Exp)
            bsum = pwrk.tile([1, 1], F32)
            nc.vector.reduce_sum(bsum[:], blendn[:], axis=AX)
            brcp = pwrk.tile([1, 1], F32)
            nc.vector.reciprocal(brcp[:], bsum[:])
            nc.vector.tensor_scalar_mul(blendn[:], blendn[:], brcp[:])
            if is_surreal:
                nc.scalar.activation(g3[:], sg_sb[:], AF.Sigmoid)
                nc.scalar.mul(g3[:], g3[:], 0.3 / (C - 1))

            # ---- logits (LS, C) = tT.T @ WcT; softmax over C (the logits
            # are O(1)-scale, so exp() is safe without max-subtraction) ----
            with tc.tile_pool(name="ps_lg", bufs=1, space="PSUM") as ps_lg:
                lg_ps = ps_lg.tile([LS, C], F32)
                for j in range(KT):
                    nc.tensor.matmul(lg_ps[:], tT_sb[:, j, :], wct_sb[:, j, :],
                                     start=(j == 0), stop=(j == KT - 1))
                e_sb = pwrk.tile([LS, C], F32)
                nc.scalar.activation(e_sb[:], lg_ps[:], AF.Exp)
            ssum = pwrk.tile([LS, 1], F32)
            nc.vector.reduce_sum(ssum[:], e_sb[:], axis=AX)
            srcp = pwrk.tile([LS, 1], F32)
            nc.vector.reciprocal(srcp[:], ssum[:])
            # cw into the batch block selected by the host-provided mask
            nc.vector.tensor_scalar(cwb[:, 0:C], e_sb[:], srcp[:],
                                    bm_sb[:, 0:1], op0=MUL, op1=MUL)
            nc.vector.tensor_scalar(cwb[:, C:2 * C], e_sb[:], srcp[:],
                                    bm_sb[:, 1:2], op0=MUL, op1=MUL)

            # ---- partial crT[d, b*c] = textN.T @ cwb; AllReduce #1 ----
            with tc.tile_pool(name="ps_cp", bufs=1, space="PSUM") as ps_cp:
                crtp_ps = ps_cp.tile([128, KT, 2 * C], F32)
                for j in range(KT):
                    nc.tensor.matmul(crtp_ps[:, j, :],
                                     tN_sb[:, 128 * j:128 * (j + 1)], cwb[:],
                                     start=True, stop=True)
                nc.vector.tensor_copy(crtp_sb[:], crtp_ps[:])
            nc.scalar.dma_start(
                out=ar1_in[:].rearrange("p (j c) -> p j c", j=KT),
                in_=crtp_sb[:])
            nc.gpsimd.collective_compute(
                "AllReduce", ADD, replica_groups=RG,
                ins=[ar1_in.opt()], outs=[ar1_out.opt()])
            nc.scalar.dma_start(
                out=crT_sb[:],
                in_=ar1_out[:].rearrange("p (j c) -> p j c",
                                         j=KT).bitcast(F32R))

            # ---- blendT column via PE transpose ----
            with tc.tile_pool(name="ps_bt", bufs=1, space="PSUM") as ps_bt:
                blt_ps = ps_bt.tile([C, 1], F32)
                nc.tensor.transpose(blt_ps[:], blendn[:], ident[0:1, 0:1])
                nc.vector.tensor_copy(blT[:], blt_ps[:])

            # ---- v[c, e] for both batches: [v_b0 | v_b1] cols ----
            with tc.tile_pool(name="ps_v", bufs=1, space="PSUM") as ps_v:
                v_ps0 = ps_v.tile([C, ES], F32)
                v_ps1 = ps_v.tile([C, ES], F32)
                for j in range(KT):
                    nc.tensor.matmul(v_ps0[:], crT_sb[:, j, 0:C],
                                     wvt_sb[:, j, :],
                                     start=(j == 0), stop=(j == KT - 1))
                    nc.tensor.matmul(v_ps1[:], crT_sb[:, j, C:2 * C],
                                     wvt_sb[:, j, :],
                                     start=(j == 0), stop=(j == KT - 1))
                nc.vector.tensor_copy(v_sb[:, 0:ES], v_ps0[:])
                nc.vector.tensor_copy(v_sb[:, ES:2 * ES], v_ps1[:])
                if is_surreal:
                    nc.vector.tensor_mul(v2_sb[:, 0:ES], v_sb[:, 0:ES],
                                         v_ps0[:])
                    nc.vector.tensor_mul(v2_sb[:, ES:2 * ES],
                                         v_sb[:, ES:2 * ES], v_ps1[:])

            # ---- fused[1, 2*ES] = blend@v + g3*(s2 - s1^2/C) ----
            with tc.tile_pool(name="ps_d", bufs=1, space="PSUM") as ps_d:
                fl_ps = ps_d.tile([1, 2 * ES], F32)
                nc.tensor.matmul(fl_ps[:], blT[:], v_sb[:],
                                 start=True, stop=True)
                if is_surreal:
                    s1_ps = ps_d.tile([1, 2 * ES], F32)
                    nc.tensor.matmul(s1_ps[:], ones6[:], v_sb[:],
                                     start=True, stop=True)
                    s2_ps = ps_d.tile([1, 2 * ES], F32)
                    nc.tensor.matmul(s2_ps[:], ones6[:], v2_sb[:],
                                     start=True, stop=True)
                    t1 = pwrk.tile([1, 2 * ES], F32)
                    nc.scalar.activation(t1[:], s1_ps[:], AF.Square)
                    t2 = pwrk.tile([1, 2 * ES], F32)
                    nc.vector.scalar_tensor_tensor(
                        t2[:], t1[:], -1.0 / C, s2_ps[:], op0=MUL, op1=ADD)
                    nc.vector.scalar_tensor_tensor(
                        fused[:], t2[:], g3[0:1, 0:1], fl_ps[:],
                        op0=MUL, op1=ADD)
                else:
                    nc.vector.tensor_copy(fused[:], fl_ps[:])

            # ---- fy = fused * nw; partial ssq per batch (both rows) ----
            nc.vector.tensor_mul(fy_sb[:], fused[:], nw_sb[:])
            nc.vector.tensor_mul(sq3[:, 0, :], fused[:, 0:ES], fused[:, 0:ES])
            nc.vector.tensor_mul(sq3[:, 1, :], fused[:, ES:2 * ES],
                                 fused[:, ES:2 * ES])
            nc.vector.reduce_sum(ssq12[:], sq3[:], axis=AX)
            nc.gpsimd.partition_broadcast(ssq2x[:], ssq12[0:1, :])
            nc.vector.tensor_copy(z_sb[:, D:D + 2], ssq2x[:])

            # ---- fyT[d-tile, b] via PE transpose ----
            with tc.tile_pool(name="ps_tr", bufs=2, space="PSUM") as ps_tr:
                for ci in range(2 * JW):
                    j, b = ci % JW, ci // JW
                    tp = ps_tr.tile([128, 1], F32, name=f"tp{ci}", tag="tp")
                    nc.tensor.transpose(tp[:],
                                        fy_sb[:, 128 * ci:128 * (ci + 1)],
                                        ident[0:1, 0:1])
                    nc.vector.tensor_copy(fyT[:, j, b:b + 1], tp[:])

            # ---- partial z[b, e'] = fyT.T @ WoT rows; AllReduce #2 ----
            with tc.tile_pool(name="ps_z", bufs=1, space="PSUM") as ps_z:
                zps = [ps_z.tile([2, 512], F32, name=f"zp{ch}", tag=f"zp{ch}")
                       for ch in range(D // 512)]
                for j in range(JW):
                    for ch in range(D // 512):
                        nc.tensor.matmul(zps[ch][:],
                                         fyT[:, j, :],
                                         wot_sb[:, j, 512 * ch:512 * (ch + 1)],
                                         start=(j == 0), stop=(j == JW - 1))
                for ch in range(D // 512):
                    nc.vector.tensor_copy(z_sb[:, 512 * ch:512 * (ch + 1)],
                                          zps[ch][:])
            nc.scalar.dma_start(out=ar2_in[:], in_=z_sb[:].bitcast(F32))
            nc.gpsimd.collective_compute(
                "AllReduce", ADD, replica_groups=RG,
                ins=[ar2_in.opt()], outs=[ar2_out.opt()])
            nc.scalar.dma_start(out=zr_sb[:], in_=ar2_out[:].bitcast(F32R))

            # ---- rsqrt(ssq/D + eps) broadcast to all partitions; the
            # scale itself is fused into the final add ----
            nc.scalar.activation(rs2[:], zr_sb[0:1, D:D + 2].bitcast(F32),
                                 AF.Sqrt, bias=eps_t[:], scale=1.0 / D)
            nc.vector.reciprocal(rs2[:], rs2[:])
            nc.gpsimd.partition_broadcast(rsB[:], rs2[:])

            # ---- broadcast z rows to 128 partitions (PSUM-resident) and
            # do the only O(N) work: out = x + z[b] * rsqrt_b ----
            with tc.tile_pool(name="ps_bc", bufs=1, space="PSUM") as ps_bc:
                bc_ps = {}
                for hb, sel in ((0, sel0r), (1, sel1r)):
                    for ch in range(D // 512):
                        t = ps_bc.tile([128, 512], F32, name=f"bc{hb}_{ch}",
                                       tag=f"bc{hb}_{ch}")
                        nc.tensor.matmul(t[:], sel[:],
                                         zr_sb[:, 512 * ch:512 * (ch + 1)],
                                         start=True, stop=True)
                        bc_ps[hb, ch] = t
                rings = [nc.scalar, nc.sync, nc.gpsimd, nc.scalar]
                for s in range(NST):
                    hb = 0 if s < NST // 2 else 1
                    for g in range(G):
                        for ch in range(D // 512):
                            sl = slice(512 * ch, 512 * (ch + 1))
                            nc.vector.scalar_tensor_tensor(
                                xts[s][:, g, sl], bc_ps[hb, ch][:],
                                rsB[:, hb:hb + 1], xts[s][:, g, sl],
                                op0=MUL, op1=ADD)
                    rings[s].dma_start(
                        out=out_d[128 * G * s:128 * G * (s + 1), :].rearrange(
                            "(g p) d -> p g d", p=128),
                        in_=xts[s][:])
            px_cm.__exit__(None, None, None)

    nc.compile()
    return nc


def prep_inputs(x, text_emb, Wc, Wv, Wo, blend_weights, surreal_gate,
                norm_weight):
    """Host-side layout prep (slice/transpose/replicate only)."""
    f = np.float32
    textN = np.asarray(text_emb, f).reshape(BL, D)
    WcT = np.ascontiguousarray(np.asarray(Wc, f).T)
    WvT = np.ascontiguousarray(np.asarray(Wv, f).T)
    WoT = np.ascontiguousarray(np.asarray(Wo, f).T)
    nw = np.asarray(norm_weight, f)
    blend = np.ascontiguousarray(np.asarray(blend_weights, f).reshape(1, C))
    sg = np.asarray(surreal_gate, f).reshape(1, 1)
    in_maps = []
    for k in range(N_CORES):
        ls = slice(LS * k, LS * (k + 1))
        es = slice(ES * k, ES * (k + 1))
        tN = np.ascontiguousarray(textN[ls])
        xs = np.concatenate(
            [x[0, HALF * k:HALF * (k + 1), :], x[1, HALF * k:HALF * (k + 1), :]],
            axis=0).astype(f)
        bm = np.zeros((LS, 2), f)
        bm[:, 0 if k < N_CORES // 2 else 1] = 1.0
        in_maps.append({
            "x_shard": np.ascontiguousarray(xs),
            "tN": tN,
            "tT": np.ascontiguousarray(tN.T),
            "WcT": WcT,
            "wvt": np.ascontiguousarray(WvT[:, es]),
            "wot": np.ascontiguousarray(WoT[es, :]),
            "blend": blend,
            "sg": sg,
            "nw2": np.concatenate([nw[es], nw[es]])[None, :].astype(f),
            "bm": bm,
        })
    return in_maps


_CACHE = {}


def kernel(x, text_emb, Wc, Wq, Wk, Wv, Wo, blend_weights, surreal_gate,
           norm_weight, is_surreal, _collect=None):
    surreal = bool(int(np.asarray(is_surreal)))
    key = ("nc", surreal)
    if key not in _CACHE:
        _CACHE[key] = build_nc(surreal)
    nc = _CACHE[key]

    in_maps = prep_inputs(x, text_emb, Wc, Wv, Wo, blend_weights,
                          surreal_gate, norm_weight)
    res = run_bass_kernel_spmd(
        nc, in_maps, core_ids=list(range(N_CORES)),
        trace=os.environ.get("KERNEL_TRACE", "0") == "1",
    )
    if _collect is not None:
        _collect.append(res)

    out = np.empty((B, N, D), np.float32)
    for k in range(N_CORES):
        shard = res.results[k]["out_shard"]
        out[0, HALF * k:HALF * (k + 1), :] = shard[:HALF]
        out[1, HALF * k:HALF * (k + 1), :] = shard[HALF:]
    return out


# revision 14
# speedup vs baseline: 1.5176x; 1.0010x over previous
"""Trainium2 Bass kernel for nn_ConceptFusionModule (8-core, 1 collective).

Math: softmax over a single key collapses the SDPA, so the module reduces to

    cw      = softmax(textN @ Wc.T, -1)           # (B*L, C)
    cr      = cw.T @ textN  (per batch)           # (B, C, D)
    v       = cr @ Wv.T                           # (B, C, D)
    fused   = blend@v + sig(g)*.3*var_c(v)        # (B, D)
    obd     = rmsnorm(fused)*nw @ Wo.T            # (B, D)
    out     = x + obd[:, None, :]                 # broadcast over N

Profiling showed the runtime's 8-core NEFF entry barrier spans a fixed
~46 us of core-launch skew, independent of kernel structure: anything a
core computes in its first ~50 us is hidden under the barrier, and every
collective op costs >=10 us of serial CC-stream time after it.  So this
version replicates the text chain (full cr from the full 8.4 MB text,
cheap, fully under the barrier) and keeps exactly ONE collective:

  - Wv: core k holds WvT[:, 256k:256k+256]  -> v/fused for its e-slice.
  - Wo: core k holds WoT[256k:256k+256, :]  -> partial z = (fused*nw)@WoT
    (z is linear in rmsnorm's input, so the global scale is applied after
    the reduce), plus partial sum(fused^2) as payload column 2048.
  - AllReduce (16.4 KB) of [z | ssq]; obd = z * rsqrt(ssq/D + eps) with
    the rsqrt folded into the PE row-broadcast selector.

Post-barrier critical path = AllReduce + sel-matmul broadcast + DVE adds
reading obd rows straight from PSUM + the 8.4 MB store stream.

Per-core HBM traffic: x 8.4 + out 8.4 + text 8.4 + Wv/8 2.1 + Wo/8 2.1
= ~29.5 MB, all loads overlapped with the entry barrier.
"""

import os

import numpy as np

import concourse.bacc as bacc
import concourse.bass as bass
import concourse.mybir as mybir
import concourse.tile as tile
from concourse import masks
from concourse.bass_utils import run_bass_kernel_spmd

F32 = mybir.dt.float32
F32R = mybir.dt.float32r

N_CORES = 8
B, N, L, D, C = 2, 4096, 256, 2048, 6
BL = B * L               # 512 text rows
LT = BL // 128           # 4 text l-tiles
ES = D // N_CORES        # 256-wide d/e slice per core
ROWS = B * N // N_CORES  # 1024 x rows per core (512 per batch)
HALF = ROWS // 2
KT = D // 128            # 16 contraction k-tiles
JW = ES // 128           # 2 k-tiles for the wot contraction
G = 2                    # 128-row tiles per x supertile
NST = ROWS // (128 * G)  # 4 supertiles
AX = mybir.AxisListType.X
AF = mybir.ActivationFunctionType
ADD = mybir.AluOpType.add
MUL = mybir.AluOpType.mult
RG = [list(range(N_CORES))]


def build_nc(is_surreal: bool) -> bacc.Bacc:
    nc = bacc.Bacc("TRN2", target_bir_lowering=False, debug=False,
                   num_devices=N_CORES)

    x_d = nc.dram_tensor("x_shard", [ROWS, D], F32, kind="ExternalInput")
    tN_d = nc.dram_tensor("tN", [BL, D], F32, kind="ExternalInput")
    tT_d = nc.dram_tensor("tT", [D, BL], F32, kind="ExternalInput")
    wct_d = nc.dram_tensor("WcT", [D, C], F32, kind="ExternalInput")
    wvt_d = nc.dram_tensor("wvt", [D, ES], F32, kind="ExternalInput")
    wot_d = nc.dram_tensor("wot", [ES, D], F32, kind="ExternalInput")
    bl_d = nc.dram_tensor("blend", [1, C], F32, kind="ExternalInput")
    sg_d = nc.dram_tensor("sg2", [2, 1], F32, kind="ExternalInput")
    nw_d = nc.dram_tensor("nw2", [2, ES], F32, kind="ExternalInput")
    out_d = nc.dram_tensor("out_shard", [ROWS, D], F32, kind="ExternalOutput")

    with tile.TileContext(nc) as tc:
        with (
            tc.tile_pool(name="pc", bufs=1) as pc,
            tc.tile_pool(name="pwrk", bufs=2) as pwrk,
            tc.tile_pool(name="dram", bufs=1, space="DRAM") as dram,
        ):
            # ---- persistent SBUF tiles ----
            tT_sb = pc.tile([128, KT, BL], F32R)
            tN_sb = pc.tile([128, LT, D], F32R)
            wct_sb = pc.tile([128, KT, C], F32R)
            wvt_sb = pc.tile([128, KT, ES], F32R)
            wot_sb = pc.tile([128, JW, D], F32R)
            bl_sb = pc.tile([1, C], F32)
            sg_sb = pc.tile([2, 1], F32)
            nw_sb = pc.tile([2, ES], F32)
            ident = pc.tile([128, 128], F32)
            eps_t = pc.tile([2, 1], F32)
            blendn = pc.tile([1, C], F32)
            blendn2 = pc.tile([1, 2 * C], F32)
            m12 = pc.tile([2 * C, 1], F32)
            m12c = pc.tile([2 * C, 1], F32)
            bd4 = pc.tile([2 * C, 4], F32)
            ones2 = pc.tile([2 * C, 2], F32)
            g3 = pc.tile([2, 1], F32)
            cwb4 = pc.tile([128, LT, 2 * C], F32R)
            crT_sb = pc.tile([128, KT, 2 * C], F32R)
            v_sb = pc.tile([2 * C, ES], F32)
            v2_sb = pc.tile([2 * C, ES], F32)
            fused = pc.tile([2, ES], F32)
            fy_sb = pc.tile([2, ES], F32)
            sqf = pc.tile([2, ES], F32)
            ssq = pc.tile([2, 1], F32)
            fyT = pc.tile([128, JW, 2], F32R)
            z_sb = pc.tile([2, D + 1], F32R)
            zr_sb = pc.tile([2, D + 1], F32R)
            ms = pc.tile([2, 1], F32)
            rs = pc.tile([2, 1], F32)
            sel0 = pc.tile([2, 128], F32)
            sel1 = pc.tile([2, 128], F32)
            sel0s = pc.tile([2, 128], F32R)
            sel1s = pc.tile([2, 128], F32R)
            bcz0 = pc.tile([128, D], F32)
            bcz1 = pc.tile([128, D], F32)

            # ---- internal DRAM (collective bounce buffers) ----
            ar2_in = dram.tile([2, D + 1], F32)
            ar2_out = dram.tile([2, D + 1], F32)

            # ---- all loads on the sync HWDGE ring, critical-path first ----
            nc.sync.dma_start(out=tT_sb[:],
                              in_=tT_d.ap().rearrange("(j p) l -> p j l",
                                                      p=128).bitcast(F32R))
            nc.sync.dma_start(out=tN_sb[:],
                              in_=tN_d.ap().rearrange("(g p) d -> p g d",
                                                      p=128).bitcast(F32R))
            nc.sync.dma_start(out=wct_sb[:],
                              in_=wct_d.ap().rearrange("(j p) c -> p j c",
                                                       p=128).bitcast(F32R))
            nc.sync.dma_start(out=bl_sb[:], in_=bl_d.ap())
            nc.sync.dma_start(out=sg_sb[:], in_=sg_d.ap())
            nc.sync.dma_start(out=nw_sb[:], in_=nw_d.ap())
            nc.sync.dma_start(out=wvt_sb[:],
                              in_=wvt_d.ap().rearrange("(j p) e -> p j e",
                                                       p=128).bitcast(F32R))
            nc.sync.dma_start(out=wot_sb[:],
                              in_=wot_d.ap().rearrange("(j p) e -> p j e",
                                                       p=128).bitcast(F32R))
            px_cm = tc.tile_pool(name="px", bufs=1)
            px = px_cm.__enter__()
            xts = []
            for s in range(NST):
                xt = px.tile([128, G, D], F32, name=f"xst{s}")
                nc.sync.dma_start(
                    out=xt[:],
                    in_=x_d[128 * G * s:128 * G * (s + 1), :].rearrange(
                        "(g p) d -> p g d", p=128))
                xts.append(xt)

            # ---- constants ----
            masks.make_identity(nc, ident[:])
            nc.gpsimd.memset(eps_t[:], 1e-6)
            nc.gpsimd.memset(cwb4[:].bitcast(F32), 0.0)
            nc.gpsimd.memset(sel0[:], 0.0)
            nc.gpsimd.memset(sel0[0:1, :], 1.0)
            nc.vector.tensor_scalar(sel1[:], sel0[:], -1.0, 1.0,
                                    op0=MUL, op1=ADD)
            # m12 = [1]*C + [0]*C column; m12c its complement
            nc.gpsimd.memset(m12[:], 0.0)
            nc.gpsimd.memset(m12[0:C, 0:1], 1.0)
            nc.vector.tensor_scalar(m12c[:], m12[:], -1.0, 1.0,
                                    op0=MUL, op1=ADD)
            nc.vector.tensor_copy(ones2[:, 0:1], m12[:])
            nc.vector.tensor_copy(ones2[:, 1:2], m12c[:])
            nc.vector.tensor_copy(bd4[:, 2:3], m12[:])
            nc.vector.tensor_copy(bd4[:, 3:4], m12c[:])

            # ---- blend softmax + g3 + bd4 blend columns (tiny, early) ----
            nc.scalar.activation(blendn[:], bl_sb[:], AF.Exp)
            bsum = pwrk.tile([1, 1], F32)
            nc.vector.reduce_sum(bsum[:], blendn[:], axis=AX)
            brcp = pwrk.tile([1, 1], F32)
            nc.vector.reciprocal(brcp[:], bsum[:])
            nc.vector.tensor_scalar_mul(blendn[:], blendn[:], brcp[:])
            nc.vector.tensor_copy(blendn2[0:1, 0:C], blendn[:])
            nc.vector.tensor_copy(blendn2[0:1, C:2 * C], blendn[:])
            with tc.tile_pool(name="ps_bl", bufs=1, space="PSUM") as ps_bl:
                blt_ps = ps_bl.tile([2 * C, 1], F32)
                nc.tensor.transpose(blt_ps[:], blendn2[:], ident[0:1, 0:1])
                nc.vector.tensor_mul(bd4[:, 0:1], blt_ps[:], m12[:])
                nc.vector.tensor_mul(bd4[:, 1:2], blt_ps[:], m12c[:])
            if is_surreal:
                nc.scalar.activation(g3[:], sg_sb[:], AF.Sigmoid)
                nc.scalar.mul(g3[:], g3[:], 0.3 / (C - 1))

            # ---- logits per l-tile; softmax over C (logits are O(1)-scale
            # with the 0.02 weight init, so exp() is safe without the max
            # subtraction) -> block-diagonal cluster weights cwb4 ----
            with tc.tile_pool(name="ps_lg", bufs=1, space="PSUM") as ps_lg:
                lg_ps = [ps_lg.tile([128, C], F32, name=f"lg{lt}",
                                    tag=f"lg{lt}") for lt in range(LT)]
                for lt in range(LT):
                    for j in range(KT):
                        nc.tensor.matmul(
                            lg_ps[lt][:],
                            tT_sb[:, j, 128 * lt:128 * (lt + 1)],
                            wct_sb[:, j, :],
                            start=(j == 0), stop=(j == KT - 1))
                for lt in range(LT):
                    e_sb = pwrk.tile([128, C], F32, name=f"e{lt}", tag="e")
                    nc.scalar.activation(e_sb[:], lg_ps[lt][:], AF.Exp)
                    ssum = pwrk.tile([128, 1], F32, name=f"ss{lt}", tag="ss")
                    nc.vector.reduce_sum(ssum[:], e_sb[:], axis=AX)
                    srcp = pwrk.tile([128, 1], F32, name=f"sr{lt}", tag="sr")
                    nc.vector.reciprocal(srcp[:], ssum[:])
                    off = 0 if lt < LT // 2 else C
                    nc.vector.tensor_scalar_mul(cwb4[:, lt, off:off + C],
                                                e_sb[:], srcp[:])

            # ---- full crT[d, b*c] = textN.T @ cwb4 (contraction over l,
            # j-sequential accumulation chains) ----
            with tc.tile_pool(name="ps_cp", bufs=1, space="PSUM") as ps_cp:
                crT_ps = ps_cp.tile([128, KT, 2 * C], F32)
                for j in range(KT):
                    for lt in range(LT):
                        nc.tensor.matmul(
                            crT_ps[:, j, :],
                            tN_sb[:, lt, 128 * j:128 * (j + 1)],
                            cwb4[:, lt, :],
                            start=(lt == 0), stop=(lt == LT - 1))
                nc.vector.tensor_copy(crT_sb[:], crT_ps[:])

            # ---- v[b*c, e-slice] = crT.T @ WvT cols ----
            with tc.tile_pool(name="ps_v", bufs=1, space="PSUM") as ps_v:
                v_ps = ps_v.tile([2 * C, ES], F32)
                for j in range(KT):
                    nc.tensor.matmul(v_ps[:], crT_sb[:, j, :], wvt_sb[:, j, :],
                                     start=(j == 0), stop=(j == KT - 1))
                nc.vector.tensor_copy(v_sb[:], v_ps[:])
                if is_surreal:
                    nc.vector.tensor_mul(v2_sb[:], v_sb[:], v_ps[:])

            # ---- fused[b, e-slice] = blend@v + g3*(s2 - s1^2/C) ----
            with tc.tile_pool(name="ps_d", bufs=1, space="PSUM") as ps_d:
                fl_ps = ps_d.tile([2, ES], F32)
                nc.tensor.matmul(fl_ps[:], bd4[:, 0:2], v_sb[:],
                                 start=True, stop=True)
                if is_surreal:
                    s1_ps = ps_d.tile([2, ES], F32)
                    nc.tensor.matmul(s1_ps[:], bd4[:, 2:4], v_sb[:],
                                     start=True, stop=True)
                    s2_ps = ps_d.tile([2, ES], F32)
                    nc.tensor.matmul(s2_ps[:], ones2[:], v2_sb[:],
                                     start=True, stop=True)
                    t1 = pwrk.tile([2, ES], F32)
                    nc.scalar.activation(t1[:], s1_ps[:], AF.Square)
                    t2 = pwrk.tile([2, ES], F32)
                    nc.vector.scalar_tensor_tensor(
                        t2[:], t1[:], -1.0 / C, s2_ps[:], op0=MUL, op1=ADD)
                    nc.vector.scalar_tensor_tensor(
                        fused[:], t2[:], g3[0:2, 0:1], fl_ps[:],
                        op0=MUL, op1=ADD)
                else:
                    nc.vector.tensor_copy(fused[:], fl_ps[:])

            # ---- fy = fused * nw; partial ssq -> payload column D ----
            nc.vector.tensor_mul(fy_sb[:], fused[:], nw_sb[:])
            nc.vector.tensor_mul(sqf[:], fused[:], fused[:])
            nc.vector.reduce_sum(ssq[:], sqf[:], axis=AX)
            nc.vector.tensor_copy(z_sb[:, D:D + 1], ssq[:])

            # ---- fyT[d-tile, b] via PE transpose ----
            with tc.tile_pool(name="ps_tr", bufs=2, space="PSUM") as ps_tr:
                for j in range(JW):
                    tp = ps_tr.tile([128, 2], F32, name=f"tp{j}", tag="tp")
                    nc.tensor.transpose(tp[:],
                                        fy_sb[:, 128 * j:128 * (j + 1)],
                                        ident[0:2, 0:2])
                    nc.vector.tensor_copy(fyT[:, j, :], tp[:])

            # ---- partial z[b, e'] = fyT.T @ WoT rows; the one AllReduce ----
            with tc.tile_pool(name="ps_z", bufs=1, space="PSUM") as ps_z:
                zps = [ps_z.tile([2, 512], F32, name=f"zp{ch}", tag=f"zp{ch}")
                       for ch in range(D // 512)]
                for j in range(JW):
                    for ch in range(D // 512):
                        nc.tensor.matmul(zps[ch][:],
                                         fyT[:, j, :],
                                         wot_sb[:, j, 512 * ch:512 * (ch + 1)],
                                         start=(j == 0), stop=(j == JW - 1))
                for ch in range(D // 512):
                    nc.vector.tensor_copy(z_sb[:, 512 * ch:512 * (ch + 1)],
                                          zps[ch][:])
            nc.scalar.dma_start(out=ar2_in[:], in_=z_sb[:].bitcast(F32))
            nc.gpsimd.collective_compute(
                "AllReduce", ADD, replica_groups=RG,
                ins=[ar2_in.opt()], outs=[ar2_out.opt()])
            nc.scalar.dma_start(out=zr_sb[:], in_=ar2_out[:].bitcast(F32R))

            # ---- rs = rsqrt(ssq/D + eps), folded into the sel rows ----
            nc.scalar.activation(ms[:], zr_sb[:, D:D + 1].bitcast(F32),
                                 AF.Sqrt, bias=eps_t[:], scale=1.0 / D)
            nc.vector.reciprocal(rs[:], ms[:])
            nc.vector.tensor_scalar_mul(sel0s[:], sel0[:], rs[:])
            nc.vector.tensor_scalar_mul(sel1s[:], sel1[:], rs[:])

            # ---- broadcast obd rows to 128 partitions (PSUM-resident) and
            # do the only O(N) work: out = x + obd[b] ----
            with tc.tile_pool(name="ps_bc", bufs=1, space="PSUM") as ps_bc:
                bc_ps = [ps_bc.tile([128, D], F32, name=f"bc{hb}",
                                    tag=f"bc{hb}") for hb in range(2)]
                for hb, sel in ((0, sel0s), (1, sel1s)):
                    for ch in range(D // 512):
                        nc.tensor.matmul(bc_ps[hb][:, 512 * ch:512 * (ch + 1)],
                                         sel[:],
                                         zr_sb[:, 512 * ch:512 * (ch + 1)],
                                         start=True, stop=True)
                # SBUF copies of the broadcast rows for the gpsimd adds
                # (gpsimd has no PSUM access); scalar engine does them.
                nc.scalar.activation(bcz0[:], bc_ps[0][:], AF.Copy)
                nc.scalar.activation(bcz1[:], bc_ps[1][:], AF.Copy)
                rings = [nc.scalar, nc.sync, nc.gpsimd, nc.scalar]
                for s in range(NST):
                    hb = 0 if s < NST // 2 else 1
                    for g in range(G):
                        t_idx = s * G + g
                        if t_idx % 4 == 3:
                            bcz = bcz0 if hb == 0 else bcz1
                            nc.gpsimd.tensor_add(xts[s][:, g, :],
                                                 xts[s][:, g, :], bcz[:])
                        else:
                            nc.vector.tensor_add(xts[s][:, g, :],
                                                 xts[s][:, g, :],
                                                 bc_ps[hb][:])
                    rings[s].dma_start(
                        out=out_d[128 * G * s:128 * G * (s + 1), :].rearrange(
                            "(g p) d -> p g d", p=128),
                        in_=xts[s][:])
            px_cm.__exit__(None, None, None)

    nc.compile()
    return nc


def prep_inputs(x, text_emb, Wc, Wv, Wo, blend_weights, surreal_gate,
                norm_weight):
    """Host-side layout prep (slice/transpose/replicate only)."""
    f = np.float32
    textN = np.ascontiguousarray(np.asarray(text_emb, f).reshape(BL, D))
    textT = np.ascontiguousarray(textN.T)
    WcT = np.ascontiguousarray(np.asarray(Wc, f).T)
    WvT = np.ascontiguousarray(np.asarray(Wv, f).T)
    WoT = np.ascontiguousarray(np.asarray(Wo, f).T)
    nw = np.asarray(norm_weight, f)
    blend = np.ascontiguousarray(np.asarray(blend_weights, f).reshape(1, C))
    sg2 = np.broadcast_to(np.asarray(surreal_gate, f).reshape(1, 1),
                          (2, 1)).copy()
    in_maps = []
    for k in range(N_CORES):
        es = slice(ES * k, ES * (k + 1))
        xs = np.concatenate(
            [x[0, HALF * k:HALF * (k + 1), :], x[1, HALF * k:HALF * (k + 1), :]],
            axis=0).astype(f)
        in_maps.append({
            "x_shard": np.ascontiguousarray(xs),
            "tN": textN,
            "tT": textT,
            "WcT": WcT,
            "wvt": np.ascontiguousarray(WvT[:, es]),
            "wot": np.ascontiguousarray(WoT[es, :]),
            "blend": blend,
            "sg2": sg2,
            "nw2": np.broadcast_to(nw[es][None, :], (2, ES)).copy(),
        })
    return in_maps


_CACHE = {}


def kernel(x, text_emb, Wc, Wq, Wk, Wv, Wo, blend_weights, surreal_gate,
           norm_weight, is_surreal, _collect=None):
    surreal = bool(int(np.asarray(is_surreal)))
    key = ("nc", surreal)
    if key not in _CACHE:
        _CACHE[key] = build_nc(surreal)
    nc = _CACHE[key]

    in_maps = prep_inputs(x, text_emb, Wc, Wv, Wo, blend_weights,
                          surreal_gate, norm_weight)
    res = run_bass_kernel_spmd(
        nc, in_maps, core_ids=list(range(N_CORES)),
        trace=os.environ.get("KERNEL_TRACE", "0") == "1",
    )
    if _collect is not None:
        _collect.append(res)

    out = np.empty((B, N, D), np.float32)
    for k in range(N_CORES):
        shard = res.results[k]["out_shard"]
        out[0, HALF * k:HALF * (k + 1), :] = shard[:HALF]
        out[1, HALF * k:HALF * (k + 1), :] = shard[HALF:]
    return out
